# revision 6
# baseline (speedup 1.0000x reference)
"""MD-LSTM (4-direction 2D LSTM) Trainium2 Bass kernel.

Sharding (8 NeuronCores, SPMD): core c handles direction (c % 4) with batch
half (c // 4) -> B_loc = 16, weights replicated per direction.

Per core the H,W recurrence runs as anti-diagonal wavefronts: 159 steps, each
computing gates for the valid diagonal cells (<=32) x 16 batch = <=512 matmul
rows, contracting [x(64); h_up(128); h_lf(128)] against [w0; u0; u1] into 5
PSUM gate tiles, then the LSTM pointwise update on ACT/DVE/GpSimd.

Diagonal state: h/c stored as (128=OC partitions, 16 guard + 32*16) SBUF,
column = guard + y*16 + b.  h_up/c_up are the same buffer read at column
offset -16 (slot y-1); guard columns stay zero; invalid slots are never
written inside the valid window so they stay zero / stale-but-unread.

Self-contained: hardcodes all shapes; reads no files.
"""
import numpy as np

import concourse.bass as bass
import concourse.bacc as bacc
import concourse.mybir as mybir
import concourse.tile as tile
from concourse import bass_utils

B, CIN, H, W, OC = 32, 64, 32, 128, 128
NSTEP = H + W - 1          # 159
BLOC = B // 2              # 16 batch per core
SW = H * BLOC              # 512 cells*batch max window
GUARD = BLOC
FP = mybir.dt.float32
BF = mybir.dt.bfloat16
F32R = mybir.dt.float32r

# gate order within the 5*OC dim after host-side reorder: [l, f, i, o, g]
# (reference order is [i, f, g, o, l]); l first = needed first in the cell
# update, g last (tanh).
GATE_ORDER = [4, 1, 0, 3, 2]       # tile j holds reference-gate GATE_ORDER[j]
J_L, J_F, J_I, J_O, J_G = 0, 1, 2, 3, 4


def _window(d):
    return max(0, d - (W - 1)), min(d, H - 1)


def build_kernel(mm_dt=BF, gate_dt=BF):
    nc = bacc.Bacc("TRN2", target_bir_lowering=False, debug=False, num_devices=8)

    x_dt = BF if mm_dt == BF else FP
    x_diag_d = nc.dram_tensor("x_diag", [CIN, NSTEP * SW], x_dt, kind="ExternalInput")
    w0_d = nc.dram_tensor("w0", [CIN, 5 * OC], FP, kind="ExternalInput")
    u0_d = nc.dram_tensor("u0", [OC, 5 * OC], FP, kind="ExternalInput")
    u1_d = nc.dram_tensor("u1", [OC, 5 * OC], FP, kind="ExternalInput")
    b_d = nc.dram_tensor("b", [OC, 5], FP, kind="ExternalInput")
    out_d = nc.dram_tensor("out_diag", [OC, NSTEP * SW], FP, kind="ExternalOutput")

    act = mybir.ActivationFunctionType
    alu = mybir.AluOpType
    h_dt = BF if mm_dt == BF else FP          # h state storage dtype
    HW_ = GUARD + SW                          # 528

    with tile.TileContext(nc) as tc:
        with (
            tc.tile_pool(name="const", bufs=1) as cpool,
            tc.tile_pool(name="state", bufs=1) as spool,
            tc.tile_pool(name="xdiag", bufs=3) as xpool,
            tc.tile_pool(name="gates", bufs=2) as gpool,
            tc.tile_pool(name="tmp", bufs=2) as tpool,
            tc.tile_pool(name="psum", bufs=8, space="PSUM") as ppool,
        ):
            # ---- weights ----
            w0s = cpool.tile([CIN, 5 * OC], FP, tag="w0")
            u0s = cpool.tile([OC, 5 * OC], FP, tag="u0")
            u1s = cpool.tile([OC, 5 * OC], FP, tag="u1")
            bs = cpool.tile([OC, 5], FP, tag="b")
            nc.sync.dma_start(w0s[:, :], w0_d.ap())
            nc.sync.dma_start(u0s[:, :], u0_d.ap())
            nc.sync.dma_start(u1s[:, :], u1_d.ap())
            nc.sync.dma_start(bs[:, :], b_d.ap())

            if mm_dt == BF:
                w0m = cpool.tile([CIN, 5 * OC], BF, tag="w0b")
                u0m = cpool.tile([OC, 5 * OC], BF, tag="u0b")
                u1m = cpool.tile([OC, 5 * OC], BF, tag="u1b")
                nc.vector.tensor_copy(w0m[:, :], w0s[:, :])
                nc.vector.tensor_copy(u0m[:, :], u0s[:, :])
                nc.vector.tensor_copy(u1m[:, :], u1s[:, :])
            elif mm_dt == F32R:
                w0m = w0s.bitcast(F32R)
                u0m = u0s.bitcast(F32R)
                u1m = u1s.bitcast(F32R)
            else:
                w0m, u0m, u1m = w0s, u0s, u1s

            # ---- state (double buffered) ----
            hb = [spool.tile([OC, HW_], h_dt, tag=f"hb{k}", name=f"hb{k}")
                  for k in range(2)]
            cb = [spool.tile([OC, HW_], FP, tag=f"cb{k}", name=f"cb{k}")
                  for k in range(2)]
            for k in range(2):
                nc.vector.memset(hb[k][:, :], 0.0)
                nc.vector.memset(cb[k][:, :], 0.0)

            def as_mm(ap):
                return ap.bitcast(F32R) if mm_dt == F32R else ap

            for d in range(NSTEP):
                cur, prev = d % 2, (d + 1) % 2
                y0, y1 = _window(d)
                nwin = (y1 - y0 + 1) * BLOC
                lo = GUARD + y0 * BLOC

                # ---- x diagonal slice (contiguous in x_diag layout) ----
                xd = xpool.tile([CIN, SW], x_dt, tag="xd", name=f"xd{d}")
                nc.sync.dma_start(
                    xd[:, 0:nwin],
                    x_diag_d.ap()[:, d * SW + y0 * BLOC: d * SW + y0 * BLOC + nwin])
                xmm = as_mm(xd[:, 0:nwin])

                rhs_up = as_mm(hb[prev][:, lo - BLOC: lo - BLOC + nwin])
                rhs_lf = as_mm(hb[prev][:, lo: lo + nwin])

                # ---- matmuls: x-projections first (h-independent), then
                # u-projections gate-by-gate so gate 0 (l) finishes first.
                ps = [ppool.tile([OC, SW], FP, tag="ps", name=f"ps{d}_{j}")
                      for j in range(5)]
                for j in range(5):
                    nc.tensor.matmul(ps[j][:, 0:nwin],
                                     w0m[:, j * OC:(j + 1) * OC], xmm,
                                     start=True, stop=False)
                for j in range(5):
                    nc.tensor.matmul(ps[j][:, 0:nwin],
                                     u0m[:, j * OC:(j + 1) * OC], rhs_up,
                                     start=False, stop=False)
                    nc.tensor.matmul(ps[j][:, 0:nwin],
                                     u1m[:, j * OC:(j + 1) * OC], rhs_lf,
                                     start=False, stop=True)

                # ---- activations (bias folded): sigmoid l,f,i,o ; tanh g
                gt = [gpool.tile([OC, SW], gate_dt, tag=f"g{j}", name=f"g{d}_{j}")
                      for j in range(5)]
                for j in [J_L, J_F, J_I, J_G, J_O]:
                    fn = act.Tanh if j == J_G else act.Sigmoid
                    nc.scalar.activation(gt[j][:, 0:nwin], ps[j][:, 0:nwin], fn,
                                         bias=bs[:, j:j + 1])

                c_up = cb[prev][:, lo - BLOC: lo - BLOC + nwin]
                c_lf = cb[prev][:, lo: lo + nwin]

                # ---- cell update ----
                # dcx = c_up - c_lf (gate-independent -> runs early)
                dcx = tpool.tile([OC, SW], FP, tag="dcx", name=f"dcx{d}")
                nc.vector.tensor_tensor(dcx[:, 0:nwin], c_up, c_lf, alu.subtract)
                # mix = l*dcx + c_lf ; then *= f
                mix = tpool.tile([OC, SW], FP, tag="mix", name=f"mix{d}")
                nc.vector.tensor_tensor(mix[:, 0:nwin], gt[J_L][:, 0:nwin],
                                        dcx[:, 0:nwin], alu.mult)
                nc.vector.tensor_tensor(mix[:, 0:nwin], mix[:, 0:nwin], c_lf,
                                        alu.add)
                # ig = i*g on gpsimd (off the DVE critical chain)
                ig = tpool.tile([OC, SW], gate_dt, tag="ig", name=f"ig{d}")
                nc.gpsimd.tensor_tensor(ig[:, 0:nwin], gt[J_I][:, 0:nwin],
                                        gt[J_G][:, 0:nwin], alu.mult)
                nc.vector.tensor_tensor(mix[:, 0:nwin], gt[J_F][:, 0:nwin],
                                        mix[:, 0:nwin], alu.mult)
                cw = cb[cur][:, lo: lo + nwin]
                nc.vector.tensor_tensor(cw, mix[:, 0:nwin], ig[:, 0:nwin],
                                        alu.add)
                # th = tanh(c_new);  h = o * th
                th = tpool.tile([OC, SW], gate_dt, tag="th", name=f"th{d}")
                nc.scalar.activation(th[:, 0:nwin], cw, act.Tanh)
                hwv = hb[cur][:, lo: lo + nwin]
                nc.vector.tensor_tensor(hwv, gt[J_O][:, 0:nwin],
                                        th[:, 0:nwin], alu.mult)

                # ---- output ----
                if h_dt == FP:
                    nc.sync.dma_start(
                        out_d.ap()[:, d * SW + y0 * BLOC: d * SW + y0 * BLOC + nwin],
                        hwv)
                else:
                    ho = tpool.tile([OC, SW], FP, tag="ho", name=f"ho{d}")
                    nc.vector.tensor_copy(ho[:, 0:nwin], hwv)
                    nc.sync.dma_start(
                        out_d.ap()[:, d * SW + y0 * BLOC: d * SW + y0 * BLOC + nwin],
                        ho[:, 0:nwin])

    nc.compile()
    return nc


_NC_CACHE = {}


def _get_nc(mm_dt, gate_dt):
    key = (str(mm_dt), str(gate_dt))
    if key not in _NC_CACHE:
        _NC_CACHE[key] = build_kernel(mm_dt, gate_dt)
    return _NC_CACHE[key]


def _flip(x, d):
    if d == 1:
        return x[:, :, :, ::-1]
    if d == 2:
        return x[:, :, ::-1, :]
    if d == 3:
        return x[:, :, ::-1, ::-1]
    return x


def _make_x_diag(x_nat):
    """(CIN,H,W,BLOC) -> (CIN, NSTEP*SW) diagonal layout, zeros elsewhere."""
    arr = np.zeros((CIN, NSTEP, H, BLOC), np.float32)
    for y in range(H):
        arr[:, y:y + W, y, :] = x_nat[:, y, :, :]
    return arr.reshape(CIN, NSTEP * SW)


def _decode(out_diag):
    """(OC, NSTEP*SW) -> (BLOC, OC, H, W)"""
    arr = out_diag.reshape(OC, NSTEP, H, BLOC)
    out = np.empty((BLOC, OC, H, W), np.float32)
    for y in range(H):
        out[:, :, y, :] = arr[:, y:y + W, y, :].transpose(2, 0, 1)
    return out


def kernel(x, w0, u0, u1, b, mm_dt=BF, gate_dt=BF, trace=False, _res=[None]):
    x = np.asarray(x, np.float32)
    w0 = np.asarray(w0, np.float32)
    u0 = np.asarray(u0, np.float32)
    u1 = np.asarray(u1, np.float32)
    b = np.asarray(b, np.float32)

    perm = np.concatenate([np.arange(g * OC, (g + 1) * OC) for g in GATE_ORDER])
    in_maps = []
    for c in range(8):
        dirn, half = c % 4, c // 4
        xs = _flip(x[half * BLOC:(half + 1) * BLOC], dirn)      # (BLOC,CIN,H,W)
        x_nat = np.ascontiguousarray(xs.transpose(1, 2, 3, 0))  # (CIN,H,W,BLOC)
        xdg = _make_x_diag(x_nat)
        if mm_dt == BF:
            import ml_dtypes
            xdg = xdg.astype(ml_dtypes.bfloat16)
        in_maps.append({
            "x_diag": xdg,
            "w0": np.ascontiguousarray(w0[dirn][:, perm]),
            "u0": np.ascontiguousarray(u0[dirn][:, perm]),
            "u1": np.ascontiguousarray(u1[dirn][:, perm]),
            "b": np.ascontiguousarray(b[dirn][perm].reshape(5, OC).T),
        })

    nc = _get_nc(mm_dt, gate_dt)
    res = bass_utils.run_bass_kernel_spmd(nc, in_maps, list(range(8)), trace=trace)
    _res[0] = res

    out = np.empty((B, 4, OC, H, W), np.float32)
    for c in range(8):
        dirn, half = c % 4, c // 4
        out[half * BLOC:(half + 1) * BLOC, dirn] = _decode(
            np.asarray(res.results[c]["out_diag"], np.float32))
    return out


# revision 9
# speedup vs baseline: 1.0988x; 1.0988x over previous
"""MD-LSTM (4-direction 2D LSTM) Trainium2 Bass kernel.

Sharding (8 NeuronCores, SPMD): core c handles direction (c % 4) with batch
half (c // 4); the 16-batch half is further split into TWO interleaved
sub-scans of 8 (A, B).  The two sub-scans are independent recurrences, so the
tensor engine runs B's matmuls while A's pointwise tail executes (and vice
versa) — keeping the PE HAM-warm and hiding the serial h->gates->h latency.

Per sub-scan the H,W recurrence runs as anti-diagonal wavefronts: 159 steps,
gates for the valid diagonal cells (<=32) x 8 batch = <=256 matmul rows,
contracting [x(64); h_up(128); h_lf(128)] against [w0; u0; u1] (bf16) into
PSUM, then the LSTM cell update on ACT/DVE/GpSimd with fp32 c-state.

State: h (bf16) / c (fp32) as (128=OC partitions, 8 guard + 32*8) SBUF,
column = guard + y*8 + b.  up-neighbors = same buffer at column offset -8
(slot y-1); guard stays zero; writes are window-restricted so invalid slots
stay zero / stale-but-unread.

Gate order in the 5*OC dim is host-reordered to [l, f, i, o, g]: l,f,i,o
(the sigmoids) occupy one (128, 1024) 2-bank PSUM tile -> ONE fused sigmoid
ACTIVATE; g (tanh) has its own tile.

Self-contained: hardcodes all shapes; reads no files.
"""
import numpy as np

import concourse.bass as bass
import concourse.bacc as bacc
import concourse.mybir as mybir
import concourse.tile as tile
from concourse import bass_utils

B, CIN, H, W, OC = 32, 64, 32, 128, 128
NSTEP = H + W - 1          # 159
BQ = 8                     # batch per sub-scan
SWQ = H * BQ               # 256 max window cells
HWQ = BQ + SWQ             # guard + slots = 264
FP = mybir.dt.float32
BF = mybir.dt.bfloat16

# host-side gate reorder: [l, f, i, o, g] (reference order [i, f, g, o, l])
GATE_ORDER = [4, 1, 0, 3, 2]
J_L, J_F, J_I, J_O, J_G = 0, 1, 2, 3, 4


def _window(d):
    return max(0, d - (W - 1)), min(d, H - 1)


def build_kernel():
    nc = bacc.Bacc("TRN2", target_bir_lowering=False, debug=False, num_devices=8)

    xs_d = [nc.dram_tensor(f"x_diag{s}", [CIN, NSTEP * SWQ], BF,
                           kind="ExternalInput") for s in range(2)]
    w0_d = nc.dram_tensor("w0", [CIN, 5 * OC], FP, kind="ExternalInput")
    u0_d = nc.dram_tensor("u0", [OC, 5 * OC], FP, kind="ExternalInput")
    u1_d = nc.dram_tensor("u1", [OC, 5 * OC], FP, kind="ExternalInput")
    b_d = nc.dram_tensor("b", [OC, 5], FP, kind="ExternalInput")
    outs_d = [nc.dram_tensor(f"out_diag{s}", [OC, NSTEP * SWQ], BF,
                             kind="ExternalOutput") for s in range(2)]

    act = mybir.ActivationFunctionType
    alu = mybir.AluOpType

    with tile.TileContext(nc) as tc:
        with (
            tc.tile_pool(name="const", bufs=1) as cpool,
            tc.tile_pool(name="state", bufs=1) as spool,
            tc.tile_pool(name="xdiag", bufs=3) as xpool,
            tc.tile_pool(name="gates", bufs=2) as gpool,
            tc.tile_pool(name="tmp", bufs=2) as tpool,
            tc.tile_pool(name="psum", bufs=2, space="PSUM") as ppool,
        ):
            # ---- weights (fp32 load -> bf16 cast once) ----
            w0s = cpool.tile([CIN, 5 * OC], FP, tag="w0")
            u0s = cpool.tile([OC, 5 * OC], FP, tag="u0")
            u1s = cpool.tile([OC, 5 * OC], FP, tag="u1")
            bs = cpool.tile([OC, 5], FP, tag="b")
            nc.sync.dma_start(w0s[:, :], w0_d.ap())
            nc.sync.dma_start(u0s[:, :], u0_d.ap())
            nc.sync.dma_start(u1s[:, :], u1_d.ap())
            nc.sync.dma_start(bs[:, :], b_d.ap())
            w0m = cpool.tile([CIN, 5 * OC], BF, tag="w0b")
            u0m = cpool.tile([OC, 5 * OC], BF, tag="u0b")
            u1m = cpool.tile([OC, 5 * OC], BF, tag="u1b")
            nc.vector.tensor_copy(w0m[:, :], w0s[:, :])
            nc.vector.tensor_copy(u0m[:, :], u0s[:, :])
            nc.vector.tensor_copy(u1m[:, :], u1s[:, :])

            # ---- per-sub-scan double-buffered state ----
            hb = [[spool.tile([OC, HWQ], BF, tag=f"hb{s}{k}", name=f"hb{s}{k}")
                   for k in range(2)] for s in range(2)]
            cb = [[spool.tile([OC, HWQ], FP, tag=f"cb{s}{k}", name=f"cb{s}{k}")
                   for k in range(2)] for s in range(2)]
            for s in range(2):
                for k in range(2):
                    nc.vector.memset(hb[s][k][:, :], 0.0)
                    nc.vector.memset(cb[s][k][:, :], 0.0)

            # per-step context holders
            class Step:
                pass

            def mm_x(s, d, st):
                """x-projection matmuls for sub-scan s at step d (h-independent)."""
                y0, y1 = _window(d)
                nwin = (y1 - y0 + 1) * BQ
                st.y0, st.nwin = y0, nwin
                st.lo = BQ + y0 * BQ
                xd = xpool.tile([CIN, SWQ], BF, tag=f"xd{s}", name=f"xd{s}_{d}")
                nc.sync.dma_start(
                    xd[:, 0:nwin],
                    xs_d[s].ap()[:, d * SWQ + y0 * BQ: d * SWQ + y0 * BQ + nwin])
                st.ps = [ppool.tile([OC, SWQ], FP, tag="ps", bufs=8,
                                    name=f"ps{s}_{d}_{j}") for j in range(5)]
                for j in range(5):
                    nc.tensor.matmul(st.ps[j][:, 0:nwin],
                                     w0m[:, j * OC:(j + 1) * OC],
                                     xd[:, 0:nwin], start=True, stop=False)

            def mm_u(s, d, st, prev):
                """h-recurrent matmuls for sub-scan s at step d."""
                nwin, lo = st.nwin, st.lo
                rhs_up = hb[s][prev][:, lo - BQ: lo - BQ + nwin]
                rhs_lf = hb[s][prev][:, lo: lo + nwin]
                for j in range(5):
                    nc.tensor.matmul(st.ps[j][:, 0:nwin],
                                     u0m[:, j * OC:(j + 1) * OC], rhs_up,
                                     start=False, stop=False)
                    nc.tensor.matmul(st.ps[j][:, 0:nwin],
                                     u1m[:, j * OC:(j + 1) * OC], rhs_lf,
                                     start=False, stop=True)

            def pointwise(s, d, st, cur, prev):
                nwin, lo = st.nwin, st.lo
                y0 = st.y0
                # fused sigmoid over l,f,i,o (bias per gate via 4 slices is
                # not expressible in one ACTIVATE -> bias folded with 4 ops
                # would cost more; instead one ACTIVATE per 2-bank tile is
                # only valid with a single per-partition bias, so use per-gate
                # slices but keep them on the same tile (fewer DVE deps).
                g4 = gpool.tile([OC, 4 * SWQ], BF, tag=f"g4{s}", name=f"g4{s}_{d}")
                gg = gpool.tile([OC, SWQ], BF, tag=f"gg{s}", name=f"gg{s}_{d}")
                for j in range(4):
                    nc.scalar.activation(g4[:, j * SWQ: j * SWQ + nwin],
                                         st.ps[j][:, 0:nwin],
                                         act.Sigmoid, bias=bs[:, j:j + 1])
                nc.scalar.activation(gg[:, 0:nwin], st.ps[J_G][:, 0:nwin],
                                     act.Tanh, bias=bs[:, J_G:J_G + 1])
                c_up = cb[s][prev][:, lo - BQ: lo - BQ + nwin]
                c_lf = cb[s][prev][:, lo: lo + nwin]
                l_ = g4[:, J_L * SWQ: J_L * SWQ + nwin]
                f_ = g4[:, J_F * SWQ: J_F * SWQ + nwin]
                i_ = g4[:, J_I * SWQ: J_I * SWQ + nwin]
                o_ = g4[:, J_O * SWQ: J_O * SWQ + nwin]
                # dcx = c_up - c_lf  (gate-independent, gpsimd)
                dcx = tpool.tile([OC, SWQ], FP, tag=f"dcx{s}", name=f"dcx{s}_{d}")
                nc.gpsimd.tensor_tensor(dcx[:, 0:nwin], c_up, c_lf, alu.subtract)
                # ig = i*g (gpsimd)
                ig = tpool.tile([OC, SWQ], BF, tag=f"ig{s}", name=f"ig{s}_{d}")
                nc.gpsimd.tensor_tensor(ig[:, 0:nwin], i_, gg[:, 0:nwin], alu.mult)
                # mix = (l*dcx + c_lf) * f ; c_new = mix + ig
                mix = tpool.tile([OC, SWQ], FP, tag=f"mix{s}", name=f"mix{s}_{d}")
                nc.vector.tensor_tensor(mix[:, 0:nwin], l_, dcx[:, 0:nwin],
                                        alu.mult)
                nc.vector.tensor_tensor(mix[:, 0:nwin], mix[:, 0:nwin], c_lf,
                                        alu.add)
                nc.vector.tensor_tensor(mix[:, 0:nwin], f_, mix[:, 0:nwin],
                                        alu.mult)
                cw = cb[s][cur][:, lo: lo + nwin]
                nc.vector.tensor_tensor(cw, mix[:, 0:nwin], ig[:, 0:nwin],
                                        alu.add)
                # th = tanh(c_new); h = o*th
                th = tpool.tile([OC, SWQ], BF, tag=f"th{s}", name=f"th{s}_{d}")
                nc.scalar.activation(th[:, 0:nwin], cw, act.Tanh)
                hwv = hb[s][cur][:, lo: lo + nwin]
                nc.vector.tensor_tensor(hwv, o_, th[:, 0:nwin], alu.mult)
                nc.sync.dma_start(
                    outs_d[s].ap()[:, d * SWQ + y0 * BQ: d * SWQ + y0 * BQ + nwin],
                    hwv)

            for d in range(NSTEP):
                cur, prev = d % 2, (d + 1) % 2
                stA, stB = Step(), Step()
                # x-projections for both sub-scans (no h dependency)
                mm_x(0, d, stA)
                mm_x(1, d, stB)
                # B's recurrent matmuls first, then A's: A's pointwise tail
                # overlaps B's matmuls and vice versa.
                mm_u(1, d, stB, prev)
                pointwise(1, d, stB, cur, prev)
                mm_u(0, d, stA, prev)
                pointwise(0, d, stA, cur, prev)

    nc.compile()
    return nc


_NC_CACHE = {}


def _get_nc():
    if "nc" not in _NC_CACHE:
        _NC_CACHE["nc"] = build_kernel()
    return _NC_CACHE["nc"]


def _flip(x, d):
    if d == 1:
        return x[:, :, :, ::-1]
    if d == 2:
        return x[:, :, ::-1, :]
    if d == 3:
        return x[:, :, ::-1, ::-1]
    return x


def _make_x_diag(x_nat):
    """(CIN,H,W,BQ) -> (CIN, NSTEP*SWQ) diagonal layout."""
    arr = np.zeros((CIN, NSTEP, H, BQ), np.float32)
    for y in range(H):
        arr[:, y:y + W, y, :] = x_nat[:, y, :, :]
    return arr.reshape(CIN, NSTEP * SWQ)


def _decode(out_diag):
    """(OC, NSTEP*SWQ) fp32 -> (BQ, OC, H, W)"""
    arr = out_diag.reshape(OC, NSTEP, H, BQ)
    out = np.empty((BQ, OC, H, W), np.float32)
    for y in range(H):
        out[:, :, y, :] = arr[:, y:y + W, y, :].transpose(2, 0, 1)
    return out


def kernel(x, w0, u0, u1, b, trace=False, _res=[None]):
    import ml_dtypes
    x = np.asarray(x, np.float32)
    w0 = np.asarray(w0, np.float32)
    u0 = np.asarray(u0, np.float32)
    u1 = np.asarray(u1, np.float32)
    b = np.asarray(b, np.float32)

    perm = np.concatenate([np.arange(g * OC, (g + 1) * OC) for g in GATE_ORDER])
    in_maps = []
    for c in range(8):
        dirn, half = c % 4, c // 4
        xs = _flip(x[half * 16:(half + 1) * 16], dirn)          # (16,CIN,H,W)
        x_nat = np.ascontiguousarray(xs.transpose(1, 2, 3, 0))  # (CIN,H,W,16)
        m = {
            "w0": np.ascontiguousarray(w0[dirn][:, perm]),
            "u0": np.ascontiguousarray(u0[dirn][:, perm]),
            "u1": np.ascontiguousarray(u1[dirn][:, perm]),
            "b": np.ascontiguousarray(b[dirn][perm].reshape(5, OC).T),
        }
        for s in range(2):
            m[f"x_diag{s}"] = _make_x_diag(
                x_nat[:, :, :, s * BQ:(s + 1) * BQ]).astype(ml_dtypes.bfloat16)
        in_maps.append(m)

    nc = _get_nc()
    res = bass_utils.run_bass_kernel_spmd(nc, in_maps, list(range(8)), trace=trace)
    _res[0] = res

    out = np.empty((B, 4, OC, H, W), np.float32)
    for c in range(8):
        dirn, half = c % 4, c // 4
        for s in range(2):
            od = np.asarray(res.results[c][f"out_diag{s}"]).astype(np.float32)
            lo = half * 16 + s * BQ
            out[lo:lo + BQ, dirn] = _decode(od)
    return out


# revision 10
# speedup vs baseline: 1.1120x; 1.0119x over previous
"""MD-LSTM (4-direction 2D LSTM) Trainium2 Bass kernel.

Sharding (8 NeuronCores, SPMD): core c handles direction (c % 4) with batch
half (c // 4); the 16-batch half is further split into TWO interleaved
sub-scans of 8 (A, B).  The two sub-scans are independent recurrences, so the
tensor engine runs B's matmuls while A's pointwise tail executes (and vice
versa) — keeping the PE HAM-warm and hiding the serial h->gates->h latency.

Per sub-scan the H,W recurrence runs as anti-diagonal wavefronts: 159 steps,
gates for the valid diagonal cells (<=32) x 8 batch = <=256 matmul rows,
contracting [x(64); h_up(128); h_lf(128)] against [w0; u0; u1] (bf16) into
PSUM, then the LSTM cell update on ACT/DVE/GpSimd with fp32 c-state.

State: h (bf16) / c (fp32) as (128=OC partitions, 8 guard + 32*8) SBUF,
column = guard + y*8 + b.  up-neighbors = same buffer at column offset -8
(slot y-1); guard stays zero; writes are window-restricted so invalid slots
stay zero / stale-but-unread.

Gate order in the 5*OC dim is host-reordered to [l, f, i, o, g]: l,f,i,o
(the sigmoids) occupy one (128, 1024) 2-bank PSUM tile -> ONE fused sigmoid
ACTIVATE; g (tanh) has its own tile.

Self-contained: hardcodes all shapes; reads no files.
"""
import numpy as np

import concourse.bass as bass
import concourse.bacc as bacc
import concourse.mybir as mybir
import concourse.tile as tile
from concourse import bass_utils

B, CIN, H, W, OC = 32, 64, 32, 128, 128
NSTEP = H + W - 1          # 159
BQ = 8                     # batch per sub-scan
SWQ = H * BQ               # 256 max window cells
HWQ = BQ + SWQ             # guard + slots = 264
FP = mybir.dt.float32
BF = mybir.dt.bfloat16

# host-side gate reorder: [l, f, i, o, g] (reference order [i, f, g, o, l])
GATE_ORDER = [4, 1, 0, 3, 2]
J_L, J_F, J_I, J_O, J_G = 0, 1, 2, 3, 4


def _window(d):
    return max(0, d - (W - 1)), min(d, H - 1)


def build_kernel():
    nc = bacc.Bacc("TRN2", target_bir_lowering=False, debug=False, num_devices=8)

    xs_d = [nc.dram_tensor(f"x_diag{s}", [CIN, NSTEP * SWQ], BF,
                           kind="ExternalInput") for s in range(2)]
    w0_d = nc.dram_tensor("w0", [CIN, 5 * OC], FP, kind="ExternalInput")
    u0_d = nc.dram_tensor("u0", [OC, 5 * OC], FP, kind="ExternalInput")
    u1_d = nc.dram_tensor("u1", [OC, 5 * OC], FP, kind="ExternalInput")
    b_d = nc.dram_tensor("b", [OC, 5], FP, kind="ExternalInput")
    outs_d = [nc.dram_tensor(f"out_diag{s}", [OC, NSTEP * SWQ], BF,
                             kind="ExternalOutput") for s in range(2)]

    act = mybir.ActivationFunctionType
    alu = mybir.AluOpType

    with tile.TileContext(nc) as tc:
        with (
            tc.tile_pool(name="const", bufs=1) as cpool,
            tc.tile_pool(name="state", bufs=1) as spool,
            tc.tile_pool(name="xdiag", bufs=3) as xpool,
            tc.tile_pool(name="gates", bufs=2) as gpool,
            tc.tile_pool(name="tmp", bufs=2) as tpool,
            tc.tile_pool(name="psum", bufs=2, space="PSUM") as ppool,
        ):
            # ---- weights (fp32 load -> bf16 cast once) ----
            w0s = cpool.tile([CIN, 5 * OC], FP, tag="w0")
            u0s = cpool.tile([OC, 5 * OC], FP, tag="u0")
            u1s = cpool.tile([OC, 5 * OC], FP, tag="u1")
            bs = cpool.tile([OC, 5], FP, tag="b")
            nc.sync.dma_start(w0s[:, :], w0_d.ap())
            nc.sync.dma_start(u0s[:, :], u0_d.ap())
            nc.sync.dma_start(u1s[:, :], u1_d.ap())
            nc.sync.dma_start(bs[:, :], b_d.ap())
            w0m = cpool.tile([CIN, 5 * OC], BF, tag="w0b")
            u0m = cpool.tile([OC, 5 * OC], BF, tag="u0b")
            u1m = cpool.tile([OC, 5 * OC], BF, tag="u1b")
            nc.vector.tensor_copy(w0m[:, :], w0s[:, :])
            nc.vector.tensor_copy(u0m[:, :], u0s[:, :])
            nc.vector.tensor_copy(u1m[:, :], u1s[:, :])

            # ---- per-sub-scan double-buffered state ----
            hb = [[spool.tile([OC, HWQ], BF, tag=f"hb{s}{k}", name=f"hb{s}{k}")
                   for k in range(2)] for s in range(2)]
            cb = [[spool.tile([OC, HWQ], FP, tag=f"cb{s}{k}", name=f"cb{s}{k}")
                   for k in range(2)] for s in range(2)]
            for s in range(2):
                for k in range(2):
                    nc.vector.memset(hb[s][k][:, :], 0.0)
                    nc.vector.memset(cb[s][k][:, :], 0.0)

            # per-step context holders
            class Step:
                pass

            def mm_x(s, d, st):
                """x-projection matmuls for sub-scan s at step d (h-independent)."""
                y0, y1 = _window(d)
                nwin = (y1 - y0 + 1) * BQ
                st.y0, st.nwin = y0, nwin
                st.lo = BQ + y0 * BQ
                xd = xpool.tile([CIN, SWQ], BF, tag=f"xd{s}", name=f"xd{s}_{d}")
                nc.sync.dma_start(
                    xd[:, 0:nwin],
                    xs_d[s].ap()[:, d * SWQ + y0 * BQ: d * SWQ + y0 * BQ + nwin])
                st.ps = [ppool.tile([OC, SWQ], FP, tag=f"ps{s}", bufs=4,
                                    name=f"ps{s}_{d}_{j}") for j in range(5)]
                for j in range(5):
                    nc.tensor.matmul(st.ps[j][:, 0:nwin],
                                     w0m[:, j * OC:(j + 1) * OC],
                                     xd[:, 0:nwin], start=True, stop=False)

            def mm_u(s, d, st, prev):
                """h-recurrent matmuls for sub-scan s at step d."""
                nwin, lo = st.nwin, st.lo
                rhs_up = hb[s][prev][:, lo - BQ: lo - BQ + nwin]
                rhs_lf = hb[s][prev][:, lo: lo + nwin]
                for j in range(5):
                    nc.tensor.matmul(st.ps[j][:, 0:nwin],
                                     u0m[:, j * OC:(j + 1) * OC], rhs_up,
                                     start=False, stop=False)
                    nc.tensor.matmul(st.ps[j][:, 0:nwin],
                                     u1m[:, j * OC:(j + 1) * OC], rhs_lf,
                                     start=False, stop=True)

            def pointwise(s, d, st, cur, prev):
                nwin, lo = st.nwin, st.lo
                y0 = st.y0
                # fused sigmoid over l,f,i,o (bias per gate via 4 slices is
                # not expressible in one ACTIVATE -> bias folded with 4 ops
                # would cost more; instead one ACTIVATE per 2-bank tile is
                # only valid with a single per-partition bias, so use per-gate
                # slices but keep them on the same tile (fewer DVE deps).
                g4 = gpool.tile([OC, 4 * SWQ], BF, tag=f"g4{s}", name=f"g4{s}_{d}")
                gg = gpool.tile([OC, SWQ], BF, tag=f"gg{s}", name=f"gg{s}_{d}")
                for j in range(4):
                    nc.scalar.activation(g4[:, j * SWQ: j * SWQ + nwin],
                                         st.ps[j][:, 0:nwin],
                                         act.Sigmoid, bias=bs[:, j:j + 1])
                nc.scalar.activation(gg[:, 0:nwin], st.ps[J_G][:, 0:nwin],
                                     act.Tanh, bias=bs[:, J_G:J_G + 1])
                c_up = cb[s][prev][:, lo - BQ: lo - BQ + nwin]
                c_lf = cb[s][prev][:, lo: lo + nwin]
                l_ = g4[:, J_L * SWQ: J_L * SWQ + nwin]
                f_ = g4[:, J_F * SWQ: J_F * SWQ + nwin]
                i_ = g4[:, J_I * SWQ: J_I * SWQ + nwin]
                o_ = g4[:, J_O * SWQ: J_O * SWQ + nwin]
                # dcx = c_up - c_lf  (gate-independent, gpsimd)
                dcx = tpool.tile([OC, SWQ], FP, tag=f"dcx{s}", name=f"dcx{s}_{d}")
                nc.gpsimd.tensor_tensor(dcx[:, 0:nwin], c_up, c_lf, alu.subtract)
                # ig = i*g (gpsimd)
                ig = tpool.tile([OC, SWQ], BF, tag=f"ig{s}", name=f"ig{s}_{d}")
                nc.gpsimd.tensor_tensor(ig[:, 0:nwin], i_, gg[:, 0:nwin], alu.mult)
                # mix = (l*dcx + c_lf) * f ; c_new = mix + ig
                mix = tpool.tile([OC, SWQ], FP, tag=f"mix{s}", name=f"mix{s}_{d}")
                nc.vector.tensor_tensor(mix[:, 0:nwin], l_, dcx[:, 0:nwin],
                                        alu.mult)
                nc.vector.tensor_tensor(mix[:, 0:nwin], mix[:, 0:nwin], c_lf,
                                        alu.add)
                nc.vector.tensor_tensor(mix[:, 0:nwin], f_, mix[:, 0:nwin],
                                        alu.mult)
                cw = cb[s][cur][:, lo: lo + nwin]
                nc.gpsimd.tensor_tensor(cw, mix[:, 0:nwin], ig[:, 0:nwin],
                                        alu.add)
                # th = tanh(c_new); h = o*th
                th = tpool.tile([OC, SWQ], BF, tag=f"th{s}", name=f"th{s}_{d}")
                nc.scalar.activation(th[:, 0:nwin], cw, act.Tanh)
                hwv = hb[s][cur][:, lo: lo + nwin]
                nc.vector.tensor_tensor(hwv, o_, th[:, 0:nwin], alu.mult)
                nc.sync.dma_start(
                    outs_d[s].ap()[:, d * SWQ + y0 * BQ: d * SWQ + y0 * BQ + nwin],
                    hwv)

            for d in range(NSTEP):
                cur, prev = d % 2, (d + 1) % 2
                stA, stB = Step(), Step()
                # x-projections for both sub-scans (no h dependency)
                mm_x(0, d, stA)
                mm_x(1, d, stB)
                # B's recurrent matmuls first, then A's: A's pointwise tail
                # overlaps B's matmuls and vice versa.
                mm_u(1, d, stB, prev)
                pointwise(1, d, stB, cur, prev)
                mm_u(0, d, stA, prev)
                pointwise(0, d, stA, cur, prev)

    nc.compile()
    return nc


_NC_CACHE = {}


def _get_nc():
    if "nc" not in _NC_CACHE:
        _NC_CACHE["nc"] = build_kernel()
    return _NC_CACHE["nc"]


def _flip(x, d):
    if d == 1:
        return x[:, :, :, ::-1]
    if d == 2:
        return x[:, :, ::-1, :]
    if d == 3:
        return x[:, :, ::-1, ::-1]
    return x


def _make_x_diag(x_nat):
    """(CIN,H,W,BQ) -> (CIN, NSTEP*SWQ) diagonal layout."""
    arr = np.zeros((CIN, NSTEP, H, BQ), np.float32)
    for y in range(H):
        arr[:, y:y + W, y, :] = x_nat[:, y, :, :]
    return arr.reshape(CIN, NSTEP * SWQ)


def _decode(out_diag):
    """(OC, NSTEP*SWQ) fp32 -> (BQ, OC, H, W)"""
    arr = out_diag.reshape(OC, NSTEP, H, BQ)
    out = np.empty((BQ, OC, H, W), np.float32)
    for y in range(H):
        out[:, :, y, :] = arr[:, y:y + W, y, :].transpose(2, 0, 1)
    return out


def kernel(x, w0, u0, u1, b, trace=False, _res=[None]):
    import ml_dtypes
    x = np.asarray(x, np.float32)
    w0 = np.asarray(w0, np.float32)
    u0 = np.asarray(u0, np.float32)
    u1 = np.asarray(u1, np.float32)
    b = np.asarray(b, np.float32)

    perm = np.concatenate([np.arange(g * OC, (g + 1) * OC) for g in GATE_ORDER])
    in_maps = []
    for c in range(8):
        dirn, half = c % 4, c // 4
        xs = _flip(x[half * 16:(half + 1) * 16], dirn)          # (16,CIN,H,W)
        x_nat = np.ascontiguousarray(xs.transpose(1, 2, 3, 0))  # (CIN,H,W,16)
        m = {
            "w0": np.ascontiguousarray(w0[dirn][:, perm]),
            "u0": np.ascontiguousarray(u0[dirn][:, perm]),
            "u1": np.ascontiguousarray(u1[dirn][:, perm]),
            "b": np.ascontiguousarray(b[dirn][perm].reshape(5, OC).T),
        }
        for s in range(2):
            m[f"x_diag{s}"] = _make_x_diag(
                x_nat[:, :, :, s * BQ:(s + 1) * BQ]).astype(ml_dtypes.bfloat16)
        in_maps.append(m)

    nc = _get_nc()
    res = bass_utils.run_bass_kernel_spmd(nc, in_maps, list(range(8)), trace=trace)
    _res[0] = res

    out = np.empty((B, 4, OC, H, W), np.float32)
    for c in range(8):
        dirn, half = c % 4, c // 4
        for s in range(2):
            od = np.asarray(res.results[c][f"out_diag{s}"]).astype(np.float32)
            lo = half * 16 + s * BQ
            out[lo:lo + BQ, dirn] = _decode(od)
    return out


# revision 11
# speedup vs baseline: 1.1662x; 1.0488x over previous
"""MD-LSTM (4-direction 2D LSTM) Trainium2 Bass kernel.

Sharding (8 NeuronCores, SPMD): core c handles direction (c % 4) with batch
half (c // 4); the 16-batch half is further split into TWO interleaved
sub-scans of 8 (A, B).  The two sub-scans are independent recurrences, so the
tensor engine runs B's matmuls while A's pointwise tail executes (and vice
versa) — keeping the PE HAM-warm and hiding the serial h->gates->h latency.

Per sub-scan the H,W recurrence runs as anti-diagonal wavefronts: 159 steps,
gates for the valid diagonal cells (<=32) x 8 batch = <=256 matmul rows,
contracting [x(64); h_up(128); h_lf(128)] against [w0; u0; u1] (bf16) into
PSUM, then the LSTM cell update on ACT/DVE/GpSimd with fp32 c-state.

State: h (bf16) / c (fp32) as (128=OC partitions, 8 guard + 32*8) SBUF,
column = guard + y*8 + b.  up-neighbors = same buffer at column offset -8
(slot y-1); guard stays zero; writes are window-restricted so invalid slots
stay zero / stale-but-unread.

Gate order in the 5*OC dim is host-reordered to [l, f, i, o, g]: l,f,i,o
(the sigmoids) occupy one (128, 1024) 2-bank PSUM tile -> ONE fused sigmoid
ACTIVATE; g (tanh) has its own tile.

Self-contained: hardcodes all shapes; reads no files.
"""
import numpy as np

import concourse.bass as bass
import concourse.bacc as bacc
import concourse.mybir as mybir
import concourse.tile as tile
from concourse import bass_utils

B, CIN, H, W, OC = 32, 64, 32, 128, 128
NSTEP = H + W - 1          # 159
BQ = 8                     # batch per sub-scan
SWQ = H * BQ               # 256 max window cells
HWQ = BQ + SWQ             # guard + slots = 264
FP = mybir.dt.float32
BF = mybir.dt.bfloat16

# host-side gate reorder: [l, f, i, o, g] (reference order [i, f, g, o, l])
GATE_ORDER = [4, 1, 0, 3, 2]
J_L, J_F, J_I, J_O, J_G = 0, 1, 2, 3, 4


def _window(d):
    return max(0, d - (W - 1)), min(d, H - 1)


def build_kernel():
    nc = bacc.Bacc("TRN2", target_bir_lowering=False, debug=False, num_devices=8)

    xs_d = [nc.dram_tensor(f"x_diag{s}", [CIN, NSTEP * SWQ], BF,
                           kind="ExternalInput") for s in range(2)]
    w0_d = nc.dram_tensor("w0", [CIN, 5 * OC], FP, kind="ExternalInput")
    u0_d = nc.dram_tensor("u0", [OC, 5 * OC], FP, kind="ExternalInput")
    u1_d = nc.dram_tensor("u1", [OC, 5 * OC], FP, kind="ExternalInput")
    b_d = nc.dram_tensor("b", [OC, 5], FP, kind="ExternalInput")
    outs_d = [nc.dram_tensor(f"out_diag{s}", [OC, NSTEP * SWQ], BF,
                             kind="ExternalOutput") for s in range(2)]

    act = mybir.ActivationFunctionType
    alu = mybir.AluOpType

    with tile.TileContext(nc) as tc:
        with (
            tc.tile_pool(name="const", bufs=1) as cpool,
            tc.tile_pool(name="state", bufs=1) as spool,
            tc.tile_pool(name="xdiag", bufs=4) as xpool,
            tc.tile_pool(name="gates", bufs=3) as gpool,
            tc.tile_pool(name="tmp", bufs=3) as tpool,
            tc.tile_pool(name="psum", bufs=2, space="PSUM") as ppool,
        ):
            # ---- weights (fp32 load -> bf16 cast once) ----
            w0s = cpool.tile([CIN, 5 * OC], FP, tag="w0")
            u0s = cpool.tile([OC, 5 * OC], FP, tag="u0")
            u1s = cpool.tile([OC, 5 * OC], FP, tag="u1")
            bs = cpool.tile([OC, 5], FP, tag="b")
            nc.sync.dma_start(w0s[:, :], w0_d.ap())
            nc.sync.dma_start(u0s[:, :], u0_d.ap())
            nc.sync.dma_start(u1s[:, :], u1_d.ap())
            nc.sync.dma_start(bs[:, :], b_d.ap())
            w0m = cpool.tile([CIN, 5 * OC], BF, tag="w0b")
            u0m = cpool.tile([OC, 5 * OC], BF, tag="u0b")
            u1m = cpool.tile([OC, 5 * OC], BF, tag="u1b")
            nc.vector.tensor_copy(w0m[:, :], w0s[:, :])
            nc.vector.tensor_copy(u0m[:, :], u0s[:, :])
            nc.vector.tensor_copy(u1m[:, :], u1s[:, :])

            # ---- per-sub-scan double-buffered state ----
            hb = [[spool.tile([OC, HWQ], BF, tag=f"hb{s}{k}", name=f"hb{s}{k}")
                   for k in range(2)] for s in range(2)]
            cb = [[spool.tile([OC, HWQ], FP, tag=f"cb{s}{k}", name=f"cb{s}{k}")
                   for k in range(2)] for s in range(2)]
            for s in range(2):
                for k in range(2):
                    nc.vector.memset(hb[s][k][:, :], 0.0)
                    nc.vector.memset(cb[s][k][:, :], 0.0)

            # per-step context holders
            class Step:
                pass

            def mm_x(s, d, st):
                """x-projection matmuls for sub-scan s at step d (h-independent)."""
                y0, y1 = _window(d)
                nwin = (y1 - y0 + 1) * BQ
                st.y0, st.nwin = y0, nwin
                st.lo = BQ + y0 * BQ
                xd = xpool.tile([CIN, SWQ], BF, tag=f"xd{s}", name=f"xd{s}_{d}")
                nc.sync.dma_start(
                    xd[:, 0:nwin],
                    xs_d[s].ap()[:, d * SWQ + y0 * BQ: d * SWQ + y0 * BQ + nwin])
                st.ps = [ppool.tile([OC, SWQ], FP, tag=f"ps{s}", bufs=4,
                                    name=f"ps{s}_{d}_{j}") for j in range(5)]
                for j in range(5):
                    nc.tensor.matmul(st.ps[j][:, 0:nwin],
                                     w0m[:, j * OC:(j + 1) * OC],
                                     xd[:, 0:nwin], start=True, stop=False)

            def mm_u(s, d, st, prev):
                """h-recurrent matmuls for sub-scan s at step d."""
                nwin, lo = st.nwin, st.lo
                rhs_up = hb[s][prev][:, lo - BQ: lo - BQ + nwin]
                rhs_lf = hb[s][prev][:, lo: lo + nwin]
                for j in range(5):
                    nc.tensor.matmul(st.ps[j][:, 0:nwin],
                                     u0m[:, j * OC:(j + 1) * OC], rhs_up,
                                     start=False, stop=False)
                    nc.tensor.matmul(st.ps[j][:, 0:nwin],
                                     u1m[:, j * OC:(j + 1) * OC], rhs_lf,
                                     start=False, stop=True)

            def pointwise(s, d, st, cur, prev):
                nwin, lo = st.nwin, st.lo
                y0 = st.y0
                # fused sigmoid over l,f,i,o (bias per gate via 4 slices is
                # not expressible in one ACTIVATE -> bias folded with 4 ops
                # would cost more; instead one ACTIVATE per 2-bank tile is
                # only valid with a single per-partition bias, so use per-gate
                # slices but keep them on the same tile (fewer DVE deps).
                g4 = gpool.tile([OC, 4 * SWQ], BF, tag=f"g4{s}", name=f"g4{s}_{d}")
                gg = gpool.tile([OC, SWQ], BF, tag=f"gg{s}", name=f"gg{s}_{d}")
                for j in (J_L, J_F):
                    nc.scalar.activation(g4[:, j * SWQ: j * SWQ + nwin],
                                         st.ps[j][:, 0:nwin],
                                         act.Sigmoid, bias=bs[:, j:j + 1])
                nc.scalar.activation(gg[:, 0:nwin], st.ps[J_G][:, 0:nwin],
                                     act.Tanh, bias=bs[:, J_G:J_G + 1])
                for j in (J_I, J_O):
                    nc.scalar.activation(g4[:, j * SWQ: j * SWQ + nwin],
                                         st.ps[j][:, 0:nwin],
                                         act.Sigmoid, bias=bs[:, j:j + 1])
                c_up = cb[s][prev][:, lo - BQ: lo - BQ + nwin]
                c_lf = cb[s][prev][:, lo: lo + nwin]
                l_ = g4[:, J_L * SWQ: J_L * SWQ + nwin]
                f_ = g4[:, J_F * SWQ: J_F * SWQ + nwin]
                i_ = g4[:, J_I * SWQ: J_I * SWQ + nwin]
                o_ = g4[:, J_O * SWQ: J_O * SWQ + nwin]
                # dcx = c_up - c_lf  (gate-independent, gpsimd)
                dcx = tpool.tile([OC, SWQ], FP, tag=f"dcx{s}", name=f"dcx{s}_{d}")
                nc.gpsimd.tensor_tensor(dcx[:, 0:nwin], c_up, c_lf, alu.subtract)
                # ig = i*g (gpsimd)
                ig = tpool.tile([OC, SWQ], BF, tag=f"ig{s}", name=f"ig{s}_{d}")
                nc.gpsimd.tensor_tensor(ig[:, 0:nwin], i_, gg[:, 0:nwin], alu.mult)
                # mix = (l*dcx + c_lf) * f ; c_new = mix + ig
                mix = tpool.tile([OC, SWQ], FP, tag=f"mix{s}", name=f"mix{s}_{d}")
                nc.vector.tensor_tensor(mix[:, 0:nwin], l_, dcx[:, 0:nwin],
                                        alu.mult)
                nc.vector.tensor_tensor(mix[:, 0:nwin], mix[:, 0:nwin], c_lf,
                                        alu.add)
                nc.vector.tensor_tensor(mix[:, 0:nwin], f_, mix[:, 0:nwin],
                                        alu.mult)
                cw = cb[s][cur][:, lo: lo + nwin]
                nc.gpsimd.tensor_tensor(cw, mix[:, 0:nwin], ig[:, 0:nwin],
                                        alu.add)
                # th = tanh(c_new); h = o*th
                th = tpool.tile([OC, SWQ], BF, tag=f"th{s}", name=f"th{s}_{d}")
                nc.scalar.activation(th[:, 0:nwin], cw, act.Tanh)
                hwv = hb[s][cur][:, lo: lo + nwin]
                nc.vector.tensor_tensor(hwv, o_, th[:, 0:nwin], alu.mult)
                nc.sync.dma_start(
                    outs_d[s].ap()[:, d * SWQ + y0 * BQ: d * SWQ + y0 * BQ + nwin],
                    hwv)

            for d in range(NSTEP):
                cur, prev = d % 2, (d + 1) % 2
                stA, stB = Step(), Step()
                # x-projections for both sub-scans (no h dependency)
                mm_x(0, d, stA)
                mm_x(1, d, stB)
                # B's recurrent matmuls first, then A's: A's pointwise tail
                # overlaps B's matmuls and vice versa.
                mm_u(1, d, stB, prev)
                pointwise(1, d, stB, cur, prev)
                mm_u(0, d, stA, prev)
                pointwise(0, d, stA, cur, prev)

    nc.compile()
    return nc


_NC_CACHE = {}


def _get_nc():
    if "nc" not in _NC_CACHE:
        _NC_CACHE["nc"] = build_kernel()
    return _NC_CACHE["nc"]


def _flip(x, d):
    if d == 1:
        return x[:, :, :, ::-1]
    if d == 2:
        return x[:, :, ::-1, :]
    if d == 3:
        return x[:, :, ::-1, ::-1]
    return x


def _make_x_diag(x_nat):
    """(CIN,H,W,BQ) -> (CIN, NSTEP*SWQ) diagonal layout."""
    arr = np.zeros((CIN, NSTEP, H, BQ), np.float32)
    for y in range(H):
        arr[:, y:y + W, y, :] = x_nat[:, y, :, :]
    return arr.reshape(CIN, NSTEP * SWQ)


def _decode(out_diag):
    """(OC, NSTEP*SWQ) fp32 -> (BQ, OC, H, W)"""
    arr = out_diag.reshape(OC, NSTEP, H, BQ)
    out = np.empty((BQ, OC, H, W), np.float32)
    for y in range(H):
        out[:, :, y, :] = arr[:, y:y + W, y, :].transpose(2, 0, 1)
    return out


def kernel(x, w0, u0, u1, b, trace=False, _res=[None]):
    import ml_dtypes
    x = np.asarray(x, np.float32)
    w0 = np.asarray(w0, np.float32)
    u0 = np.asarray(u0, np.float32)
    u1 = np.asarray(u1, np.float32)
    b = np.asarray(b, np.float32)

    perm = np.concatenate([np.arange(g * OC, (g + 1) * OC) for g in GATE_ORDER])
    in_maps = []
    for c in range(8):
        dirn, half = c % 4, c // 4
        xs = _flip(x[half * 16:(half + 1) * 16], dirn)          # (16,CIN,H,W)
        x_nat = np.ascontiguousarray(xs.transpose(1, 2, 3, 0))  # (CIN,H,W,16)
        m = {
            "w0": np.ascontiguousarray(w0[dirn][:, perm]),
            "u0": np.ascontiguousarray(u0[dirn][:, perm]),
            "u1": np.ascontiguousarray(u1[dirn][:, perm]),
            "b": np.ascontiguousarray(b[dirn][perm].reshape(5, OC).T),
        }
        for s in range(2):
            m[f"x_diag{s}"] = _make_x_diag(
                x_nat[:, :, :, s * BQ:(s + 1) * BQ]).astype(ml_dtypes.bfloat16)
        in_maps.append(m)

    nc = _get_nc()
    res = bass_utils.run_bass_kernel_spmd(nc, in_maps, list(range(8)), trace=trace)
    _res[0] = res

    out = np.empty((B, 4, OC, H, W), np.float32)
    for c in range(8):
        dirn, half = c % 4, c // 4
        for s in range(2):
            od = np.asarray(res.results[c][f"out_diag{s}"]).astype(np.float32)
            lo = half * 16 + s * BQ
            out[lo:lo + BQ, dirn] = _decode(od)
    return out
